# revision 1
# baseline (speedup 1.0000x reference)
"""Trainium2 Bass kernel for nn_ClinicalEmbedding (EmbeddingBag-style ragged gather).

Semantics (matches reference.py):
  flat = codes.reshape(B, L); g = renorm(W[flat])  (max_norm=1.0)
  out[b, v] = 0                       for v <  V - nv[b]
            = g[b, v - (V-nv[b])]     for V-nv[b] <= v < V-1
            = sum_{j=nv-1}^{nv*C-1} g[b, j]   for v = V-1

Sharding: data-parallel over batch across 8 cores, W replicated.
All data-dependent structure (gather indices, weights/masks) is expressed as
per-core *data*, so a single NEFF runs SPMD on all 8 cores. Chunk counts are
balanced across cores by sorting patients by bag length and snake-dealing.
"""

import math
import os

import numpy as np

import concourse.bacc as bacc
import concourse.bass as bass
import concourse.mybir as mybir
import concourse.tile as tile
from concourse.bass_utils import run_bass_kernel_spmd

P = 128          # SBUF partitions
N_CORES = 8

# group size (chunks of 128 gathered rows) per indirect DMA in the bag pass
G = 13

LAST_RESULTS = None   # test harness reads profiling info from here


def _prepare(codes, nv, B, V, C, L, VOCAB, E):
    """Host-side index/mask construction. Returns static structure + per-core data."""
    B_LOC = B // N_CORES
    nbag = nv * (C - 1) + 1                    # bag length per patient
    nch = (nbag + P - 1) // P                  # gather chunks per patient
    order = np.argsort(-nch, kind="stable")    # rank -> patient, desc by work

    assign = np.empty((N_CORES, B_LOC), dtype=np.int64)
    for r, b in enumerate(order):
        assign[r % N_CORES, r // N_CORES] = b

    # static per-slot chunk count = max over cores = first patient of each rank-group
    S = np.array([nch[order[s * N_CORES]] for s in range(B_LOC)], dtype=np.int64)
    offs = np.concatenate([[0], np.cumsum(S)]).astype(np.int64)
    T = int(offs[-1])
    slot_of_chunk = np.repeat(np.arange(B_LOC), S)

    # ---- bag pass data: idxB [P, T] int32, wB [P, T] f32 ----
    idxB = np.zeros((N_CORES, T, P), np.int32)
    wB = np.zeros((N_CORES, T, P), np.float32)
    for k in range(N_CORES):
        for s in range(B_LOC):
            b = assign[k, s]
            n = int(nv[b])
            nb = n * (C - 1) + 1
            vals = codes[b, n - 1 : n - 1 + nb]
            c0, c1 = offs[s], offs[s + 1]
            cap = int(c1 - c0) * P
            bi = np.zeros(cap, np.int32)
            bw = np.zeros(cap, np.float32)
            bi[:nb] = vals
            bw[:nb] = 1.0
            idxB[k, c0:c1, :] = bi.reshape(-1, P)
            wB[k, c0:c1, :] = bw.reshape(-1, P)
    idxB = np.ascontiguousarray(idxB.transpose(0, 2, 1))   # [cores, P, T]
    wB = np.ascontiguousarray(wB.transpose(0, 2, 1))

    # ---- singles pass data: idxS [P, SC] int32, wS [P, SC] f32 ----
    # entry (p, c): p = b0*(V-1) + v for b0 in {0,1}, v in [0, V-1); slot = 2c + b0
    SC = B_LOC // 2
    idxS = np.zeros((N_CORES, P, SC), np.int32)
    wS = np.zeros((N_CORES, P, SC), np.float32)
    v_arr = np.arange(V - 1)
    for k in range(N_CORES):
        for s in range(B_LOC):
            b = assign[k, s]
            n = int(nv[b])
            b0, c = s % 2, s // 2
            valid = v_arr >= (V - n)
            j = np.clip(v_arr - (V - n), 0, L - 1)
            idxS[k, b0 * (V - 1) + v_arr, c] = np.where(valid, codes[b, j], 0)
            wS[k, b0 * (V - 1) + v_arr, c] = valid.astype(np.float32)

    return dict(
        B_LOC=B_LOC, T=T, offs=offs, slot_of_chunk=slot_of_chunk, SC=SC,
        assign=assign, idxB=idxB, wB=wB, idxS=idxS, wS=wS,
    )


def _build(prep, V, C, VOCAB, E):
    """Emit the Bass/Tile program (shared across all 8 cores)."""
    B_LOC, T, offs, soc, SC = (
        prep["B_LOC"], prep["T"], prep["offs"], prep["slot_of_chunk"], prep["SC"]
    )
    f32 = mybir.dt.float32
    i32 = mybir.dt.int32

    nc = bacc.Bacc("TRN2", num_devices=N_CORES, debug=False)
    W_d = nc.dram_tensor("W", [VOCAB, E], f32, kind="ExternalInput")
    idxB_d = nc.dram_tensor("idxB", [P, T], i32, kind="ExternalInput")
    wB_d = nc.dram_tensor("wB", [P, T], f32, kind="ExternalInput")
    idxS_d = nc.dram_tensor("idxS", [P, SC], i32, kind="ExternalInput")
    wS_d = nc.dram_tensor("wS", [P, SC], f32, kind="ExternalInput")
    out_d = nc.dram_tensor("out", [B_LOC * V, E], f32, kind="ExternalOutput")

    n_groups = math.ceil(T / G)

    with tile.TileContext(nc) as tc:
        with (
            tc.tile_pool(name="const", bufs=1) as cpool,
            tc.tile_pool(name="g", bufs=3) as gpool,
            tc.tile_pool(name="sq", bufs=2) as sqpool,
            tc.tile_pool(name="sm", bufs=2) as smpool,
            tc.tile_pool(name="ps", bufs=1, space="PSUM") as pspool,
        ):
            idxB_t = cpool.tile_from(idxB_d[:])
            wB_t = cpool.tile_from(wB_d[:])
            idxS_t = cpool.tile_from(idxS_d[:])
            wS_t = cpool.tile_from(wS_d[:])

            psum = pspool.tile([1, B_LOC * E], f32)

            # zero bias tile written by DVE so ACT sqrt waits only on DVE
            zbias = smpool.tile([P, 1], f32, tag="zbias", bufs=1)
            nc.vector.memset(zbias[:], 0.0)

            # ---------------- singles pass ----------------
            gS = gpool.tile([P, SC * E], f32, tag="gS", bufs=1)
            for c in range(SC):
                nc.gpsimd.indirect_dma_start(
                    out=gS[:, c * E : (c + 1) * E], out_offset=None, in_=W_d[:],
                    in_offset=bass.IndirectOffsetOnAxis(ap=idxS_t[:, c : c + 1], axis=0),
                )
            sqS = sqpool.tile([P, SC * E], f32, tag="sqS", bufs=1)
            nc.vector.tensor_mul(sqS[:], gS[:], gS[:])
            nS = smpool.tile([P, SC], f32, tag="nS", bufs=1)
            nc.vector.tensor_reduce(
                nS[:], sqS[:].rearrange("p (c e) -> p c e", e=E),
                axis=mybir.AxisListType.X, op=mybir.AluOpType.add,
            )
            nc.vector.tensor_scalar_max(nS[:], nS[:], 1.0)
            sqS2 = smpool.tile([P, SC], f32, tag="sqS2", bufs=1)
            nc.scalar.activation(
                sqS2[:], nS[:], mybir.ActivationFunctionType.Sqrt, bias=zbias[:]
            )
            rS = smpool.tile([P, SC], f32, tag="rS", bufs=1)
            nc.vector.reciprocal(rS[:], sqS2[:])
            nc.vector.tensor_mul(rS[:], rS[:], wS_t[:])
            nc.vector.tensor_tensor(
                out=gS[:].rearrange("p (c e) -> p c e", e=E),
                in0=gS[:].rearrange("p (c e) -> p c e", e=E),
                in1=rS[:].to_broadcast([P, SC, E]),
                op=mybir.AluOpType.mult,
            )
            # store rows (slot=2c+b0, v) <- gS[p=b0*(V-1)+v, block c]
            out_bv = out_d[:].rearrange("(c b0 v) e -> b0 v c e", c=SC, b0=2, v=V)
            for b0 in range(2):
                nc.sync.dma_start(
                    out=out_bv[b0, : V - 1],
                    in_=gS[b0 * (V - 1) : (b0 + 1) * (V - 1), :].rearrange(
                        "p (c e) -> p c e", e=E
                    ),
                )

            # ---------------- bag pass ----------------
            for g in range(n_groups):
                c0, c1 = g * G, min((g + 1) * G, T)
                Gg = c1 - c0
                gB = gpool.tile([P, Gg * E], f32, tag="gB")
                for cl in range(Gg):
                    nc.gpsimd.indirect_dma_start(
                        out=gB[:, cl * E : (cl + 1) * E], out_offset=None, in_=W_d[:],
                        in_offset=bass.IndirectOffsetOnAxis(
                            ap=idxB_t[:, c0 + cl : c0 + cl + 1], axis=0
                        ),
                    )
                sqB = sqpool.tile([P, Gg * E], f32, tag="sqB")
                nc.vector.tensor_mul(sqB[:], gB[:], gB[:])
                nB = smpool.tile([P, Gg], f32, tag="nB")
                nc.vector.tensor_reduce(
                    nB[:], sqB[:].rearrange("p (c e) -> p c e", e=E),
                    axis=mybir.AxisListType.X, op=mybir.AluOpType.add,
                )
                nc.vector.tensor_scalar_max(nB[:], nB[:], 1.0)
                sqB2 = smpool.tile([P, Gg], f32, tag="sqB2")
                nc.scalar.activation(
                    sqB2[:], nB[:], mybir.ActivationFunctionType.Sqrt, bias=zbias[:]
                )
                rB = smpool.tile([P, Gg], f32, tag="rB")
                nc.vector.reciprocal(rB[:], sqB2[:])
                nc.vector.tensor_mul(rB[:], rB[:], wB_t[:, c0:c1])
                for c in range(c0, c1):
                    s = int(soc[c])
                    cl = c - c0
                    nc.tensor.matmul(
                        out=psum[0:1, s * E : (s + 1) * E],
                        lhsT=rB[:, cl : cl + 1],
                        rhs=gB[:, cl * E : (cl + 1) * E],
                        start=(c == offs[s]),
                        stop=(c == offs[s + 1] - 1),
                    )

            outS = smpool.tile([1, B_LOC * E], f32, tag="outS", bufs=1)
            nc.vector.tensor_copy(outS[:], psum[:])
            nc.sync.dma_start(
                out=out_d[:].rearrange("(b v) e -> b v e", v=V)[:, V - 1, :],
                in_=outS[:].rearrange("p (b e) -> p b e", e=E),
            )

    nc.compile()   # bacc passes: wait-splitting (<=1 wait/instr on TRN2), nop fusion
    return nc


def kernel(**inputs) -> np.ndarray:
    global LAST_RESULTS
    W = np.ascontiguousarray(np.asarray(inputs["W"], dtype=np.float32))
    codes_in = np.asarray(inputs["codes"])
    nv = np.asarray(inputs["n_visits"]).astype(np.int64)

    B, V, C = codes_in.shape
    VOCAB, E = W.shape
    L = V * C
    codes = np.ascontiguousarray(codes_in.reshape(B, L).astype(np.int32))

    prep = _prepare(codes, nv, B, V, C, L, VOCAB, E)
    nc = _build(prep, V, C, VOCAB, E)

    in_maps = [
        {
            "W": W,
            "idxB": prep["idxB"][k],
            "wB": prep["wB"][k],
            "idxS": prep["idxS"][k],
            "wS": prep["wS"][k],
        }
        for k in range(N_CORES)
    ]
    trace = bool(int(os.environ.get("KERNEL_TRACE", "0")))
    res = run_bass_kernel_spmd(
        nc, in_maps, core_ids=list(range(N_CORES)), trace=trace
    )
    LAST_RESULTS = res

    B_LOC = prep["B_LOC"]
    assign = prep["assign"]
    full = np.zeros((B, V, E), np.float32)
    for k in range(N_CORES):
        o = res.results[k]["out"].reshape(B_LOC, V, E)
        full[assign[k]] = o
    return full



# revision 4
# speedup vs baseline: 1.4272x; 1.4272x over previous
"""Trainium2 Bass kernel for nn_ClinicalEmbedding (EmbeddingBag-style ragged gather).

Semantics (matches reference.py):
  flat = codes.reshape(B, L); g = renorm(W[flat])  (max_norm=1.0)
  out[b, v] = 0                       for v <  V - nv[b]
            = g[b, v - (V-nv[b])]     for V-nv[b] <= v < V-1
            = sum_{j=nv-1}^{nv*C-1} g[b, j]   for v = V-1

Sharding: data-parallel over batch across 8 cores, W replicated.

Device strategy (v3): the bag sum for slot s is
    sum_u CNT[u, s] * rsqrt(max(1, |W_u|^2)) * W_u
over the per-core set of UNIQUE bag codes u. Unique rows are fetched with
4 large dma_gather calls (int16 indices -> vocab split into 4x32768-row
buckets; np.unique gives sorted uniques so buckets are contiguous runs).
This replaces ~200 per-chunk SWDGE indirect DMAs (~1us fixed cost each)
with 4 instructions. Renorm factors come from ACT square + DVE reduce;
the host-built count matrix is scaled by them and one 128x32 @ 128xE fp32
matmul per chunk accumulates bag sums in PSUM. Singles (individual visit
rows) are gathered position-mapped with per-column indirect DMAs so the
renormalized rows store out with two plain strided DMAs.
"""

import math
import os

import numpy as np

import concourse.bacc as bacc
import concourse.bass as bass
import concourse.mybir as mybir
import concourse.tile as tile
from concourse.bass_utils import run_bass_kernel_spmd

P = 128          # SBUF partitions
N_CORES = 8
GCH = 25         # chunks per compute block
BUCK = 1 << 15   # vocab rows per dma_gather bucket (int16 index range)

LAST_RESULTS = None   # test harness reads profiling info from here


def _prepare(codes, nv, B, V, C, L, VOCAB, E):
    """Host-side index/count construction. Returns static structure + per-core data."""
    B_LOC = B // N_CORES
    NBUCK = (VOCAB + BUCK - 1) // BUCK
    nbag = nv * (C - 1) + 1                    # bag length per patient

    # balanced LPT assignment: sort desc by bag length, give to least-loaded core
    order = np.argsort(-nbag, kind="stable")
    loads = np.zeros(N_CORES, dtype=np.int64)
    counts = np.zeros(N_CORES, dtype=np.int64)
    assign = np.zeros((N_CORES, B_LOC), dtype=np.int64)
    for b in order:
        k = min((kk for kk in range(N_CORES) if counts[kk] < B_LOC),
                key=lambda kk: loads[kk])
        assign[k, counts[k]] = b
        counts[k] += 1
        loads[k] += nbag[b]

    # ---- unique bag codes + per-bucket counts per core ----
    uniqs, cnts, bsizes = [], [], []
    for k in range(N_CORES):
        vals_l, slots_l = [], []
        for s in range(B_LOC):
            b = assign[k, s]
            n = int(nv[b])
            vals_l.append(codes[b, n - 1 : n * C])
            slots_l.append(np.full(n * (C - 1) + 1, s, dtype=np.int64))
        vals = np.concatenate(vals_l)
        slots = np.concatenate(slots_l)
        uniq, inv = np.unique(vals, return_inverse=True)   # sorted -> bucket runs
        cnt = np.zeros((len(uniq), B_LOC), np.float32)
        np.add.at(cnt, (inv, slots), 1.0)
        uniqs.append(uniq)
        cnts.append(cnt)
        bsizes.append(np.bincount(uniq // BUCK, minlength=NBUCK))

    # static per-bucket sizes: max over cores, rounded to 128
    S = np.array([
        -(-max(bs[j] for bs in bsizes) // P) * P for j in range(NBUCK)
    ], dtype=np.int64)
    offs = np.concatenate([[0], np.cumsum(S)]).astype(np.int64)
    NB = int(offs[-1])
    T_BAG = NB // P
    SC = B_LOC // 2                            # singles columns (2 patients each)
    T_ALL = SC + T_BAG

    idxS = np.zeros((N_CORES, P, SC), np.int32)
    wS = np.zeros((N_CORES, P, SC), np.float32)
    idx16 = np.zeros((N_CORES, P, NB // 16), np.int16)
    CNT = np.zeros((N_CORES, P, T_BAG * B_LOC), np.float32)
    v_arr = np.arange(V - 1)

    for k in range(N_CORES):
        # singles: partition p = b0*(V-1)+v, column c; patient slot s = 2c+b0
        for s in range(B_LOC):
            b = assign[k, s]
            n = int(nv[b])
            b0, c = s % 2, s // 2
            valid = v_arr >= (V - n)
            j = np.clip(v_arr - (V - n), 0, L - 1)
            idxS[k, b0 * (V - 1) + v_arr, c] = np.where(valid, codes[b, j], 0)
            wS[k, b0 * (V - 1) + v_arr, c] = valid.astype(np.float32)
        # bag: bucketed flat stream; element i -> (p=i%128, t=i//128)
        u = uniqs[k]
        flat = np.zeros(NB, np.int64)          # local (in-bucket) row ids
        cp = np.zeros((NB, B_LOC), np.float32)
        pos = 0
        for j in range(NBUCK):
            seg = u[(u >= j * BUCK) & (u < (j + 1) * BUCK)]
            mask = (u >= j * BUCK) & (u < (j + 1) * BUCK)
            o = int(offs[j])
            flat[o : o + len(seg)] = seg - j * BUCK
            cp[o : o + len(seg)] = cnts[k][mask]
            pos += len(seg)
        assert flat.max() < BUCK
        # int16 wrap: element i at (i%16 + 16*rep, i//16)
        wrap = flat.astype(np.int16).reshape(NB // 16, 16).T    # [16, NB/16]
        idx16[k] = np.tile(wrap, (8, 1))
        # [NB, 32] -> [128, T_BAG*32] with (p=i%128, t=i//128)
        CNT[k] = cp.reshape(T_BAG, P, B_LOC).transpose(1, 0, 2).reshape(
            P, T_BAG * B_LOC
        )

    return dict(
        B_LOC=B_LOC, T_BAG=T_BAG, SC=SC, T_ALL=T_ALL, S=S, offs=offs,
        NBUCK=NBUCK, assign=assign, idxS=idxS, wS=wS, idx16=idx16, CNT=CNT,
    )


def _build(prep, V, C, VOCAB, E):
    """Emit the Bass/Tile program (shared across all 8 cores)."""
    B_LOC, T_BAG, SC, T_ALL = (
        prep["B_LOC"], prep["T_BAG"], prep["SC"], prep["T_ALL"]
    )
    S, offs, NBUCK = prep["S"], prep["offs"], prep["NBUCK"]
    NB = T_BAG * P
    f32 = mybir.dt.float32
    i32 = mybir.dt.int32
    i16 = mybir.dt.int16

    nc = bacc.Bacc("TRN2", num_devices=N_CORES, debug=False)
    W_d = nc.dram_tensor("W", [VOCAB, E], f32, kind="ExternalInput")
    idxS_d = nc.dram_tensor("idxS", [P, SC], i32, kind="ExternalInput")
    wS_d = nc.dram_tensor("wS", [P, SC], f32, kind="ExternalInput")
    idx16_d = nc.dram_tensor("idx16", [P, NB // 16], i16, kind="ExternalInput")
    cnt_d = nc.dram_tensor("CNT", [P, T_BAG * B_LOC], f32, kind="ExternalInput")
    out_d = nc.dram_tensor("out", [B_LOC * V, E], f32, kind="ExternalOutput")

    # compute blocks: bag in GCH-sized chunk blocks, then singles
    blocks = []
    c = SC
    while c < T_ALL:
        blocks.append((c, min(c + GCH, T_ALL)))
        c = min(c + GCH, T_ALL)
    blocks.append((0, SC))

    with tile.TileContext(nc) as tc:
        with (
            tc.tile_pool(name="const", bufs=1) as cpool,
            tc.tile_pool(name="g", bufs=1) as gpool,
            tc.tile_pool(name="sq", bufs=2) as sqpool,
            tc.tile_pool(name="sm", bufs=2) as smpool,
            tc.tile_pool(name="ps", bufs=1, space="PSUM") as pspool,
        ):
            idxS_t = cpool.tile_from(idxS_d[:])
            wS_t = cpool.tile_from(wS_d[:])
            idx16_t = cpool.tile_from(idx16_d[:])
            cnt_t = cpool.tile_from(cnt_d[:])

            g = gpool.tile([P, T_ALL * E], f32, tag="g", bufs=1)
            n_t = smpool.tile([P, T_ALL], f32, tag="n", bufs=1)
            rn = smpool.tile([P, T_ALL], f32, tag="rn", bufs=1)
            M = gpool.tile([P, T_BAG * B_LOC], f32, tag="M", bufs=1)
            psum = pspool.tile([B_LOC, E], f32)

            # zero bias tile written by DVE so ACT waits only on DVE
            zbias = smpool.tile([P, 1], f32, tag="zbias", bufs=1)
            nc.vector.memset(zbias[:], 0.0)

            # ---- bag gathers: one dma_gather per vocab bucket ----
            for j in range(NBUCK):
                nj = int(S[j])
                if nj == 0:
                    continue
                o = int(offs[j])
                nrows = min(VOCAB, (j + 1) * BUCK) - j * BUCK
                nc.gpsimd.dma_gather(
                    out_ap=g[:, (SC + o // P) * E : (SC + (o + nj) // P) * E]
                    .rearrange("p (c e) -> p c e", e=E),
                    in_ap=W_d[j * BUCK : j * BUCK + nrows],
                    idxs_ap=idx16_t[:, o // 16 : (o + nj) // 16],
                    num_idxs=nj,
                    num_idxs_reg=nj,
                    elem_size=E,
                    single_packet=False,
                )
            # ---- singles gathers: per-column indirect DMA ----
            for c in range(SC):
                nc.gpsimd.indirect_dma_start(
                    out=g[:, c * E : (c + 1) * E],
                    out_offset=None,
                    in_=W_d[:],
                    in_offset=bass.IndirectOffsetOnAxis(
                        ap=idxS_t[:, c : c + 1], axis=0
                    ),
                )

            # ---- per-block compute ----
            for (c0, c1) in blocks:
                w = c1 - c0
                sq = sqpool.tile([P, GCH * E], f32, tag="sq")
                nc.scalar.activation(
                    sq[:, : w * E], g[:, c0 * E : c1 * E],
                    mybir.ActivationFunctionType.Square, bias=zbias[:],
                )
                nc.vector.tensor_reduce(
                    n_t[:, c0:c1],
                    sq[:, : w * E].rearrange("p (c e) -> p c e", e=E),
                    axis=mybir.AxisListType.X, op=mybir.AluOpType.add,
                )
                nc.vector.tensor_scalar_max(n_t[:, c0:c1], n_t[:, c0:c1], 1.0)
                nc.scalar.activation(
                    rn[:, c0:c1], n_t[:, c0:c1],
                    mybir.ActivationFunctionType.Sqrt, bias=zbias[:],
                )
                nc.vector.reciprocal(rn[:, c0:c1], rn[:, c0:c1])

                if c0 == 0:
                    # singles: scale rows, store with two strided DMAs
                    nc.vector.tensor_mul(rn[:, 0:SC], rn[:, 0:SC], wS_t[:])
                    nc.vector.tensor_tensor(
                        out=g[:, : SC * E].rearrange("p (c e) -> p c e", e=E),
                        in0=g[:, : SC * E].rearrange("p (c e) -> p c e", e=E),
                        in1=rn[:, 0:SC].to_broadcast([P, SC, E]),
                        op=mybir.AluOpType.mult,
                    )
                    out_bv = out_d[:].rearrange(
                        "(c two v) e -> two v c e", two=2, v=V
                    )
                    for b0 in range(2):
                        nc.sync.dma_start(
                            out=out_bv[b0, : V - 1],
                            in_=g[b0 * (V - 1) : (b0 + 1) * (V - 1), : SC * E]
                            .rearrange("p (c e) -> p c e", e=E),
                        )
                else:
                    t0, t1 = c0 - SC, c1 - SC
                    nc.vector.tensor_tensor(
                        out=M[:, t0 * B_LOC : t1 * B_LOC].rearrange(
                            "p (c s) -> p c s", s=B_LOC
                        ),
                        in0=cnt_t[:, t0 * B_LOC : t1 * B_LOC].rearrange(
                            "p (c s) -> p c s", s=B_LOC
                        ),
                        in1=rn[:, c0:c1].to_broadcast([P, w, B_LOC]),
                        op=mybir.AluOpType.mult,
                    )
                    for t in range(t0, t1):
                        nc.tensor.matmul(
                            out=psum[:, :],
                            lhsT=M[:, t * B_LOC : (t + 1) * B_LOC],
                            rhs=g[:, (SC + t) * E : (SC + t + 1) * E],
                            start=(t == 0),
                            stop=(t == T_BAG - 1),
                        )

            outS = smpool.tile([B_LOC, E], f32, tag="outS", bufs=1)
            nc.vector.tensor_copy(outS[:], psum[:])
            nc.sync.dma_start(
                out=out_d[:].rearrange("(s v) e -> s v e", v=V)[:, V - 1, :],
                in_=outS[:],
            )

    nc.compile()
    return nc


def kernel(**inputs) -> np.ndarray:
    global LAST_RESULTS
    W = np.ascontiguousarray(np.asarray(inputs["W"], dtype=np.float32))
    codes_in = np.asarray(inputs["codes"])
    nv = np.asarray(inputs["n_visits"]).astype(np.int64)

    B, V, C = codes_in.shape
    VOCAB, E = W.shape
    L = V * C
    codes = np.ascontiguousarray(codes_in.reshape(B, L).astype(np.int32))

    prep = _prepare(codes, nv, B, V, C, L, VOCAB, E)
    nc = _build(prep, V, C, VOCAB, E)

    in_maps = [
        {
            "W": W,
            "idxS": prep["idxS"][k],
            "wS": prep["wS"][k],
            "idx16": prep["idx16"][k],
            "CNT": prep["CNT"][k],
        }
        for k in range(N_CORES)
    ]
    trace = bool(int(os.environ.get("KERNEL_TRACE", "0")))
    res = run_bass_kernel_spmd(
        nc, in_maps, core_ids=list(range(N_CORES)), trace=trace
    )
    LAST_RESULTS = res

    B_LOC = prep["B_LOC"]
    assign = prep["assign"]
    full = np.zeros((B, V, E), np.float32)
    for k in range(N_CORES):
        o = res.results[k]["out"].reshape(B_LOC, V, E)
        full[assign[k]] = o
    return full


# revision 10
# speedup vs baseline: 3.2049x; 2.2456x over previous
"""Trainium2 Bass kernel for nn_ClinicalEmbedding (EmbeddingBag-style ragged gather).

Semantics (matches reference.py):
  flat = codes.reshape(B, L); g = renorm(W[flat])  (max_norm=1.0)
  out[b, v] = 0                       for v <  V - nv[b]
            = g[b, v - (V-nv[b])]     for V-nv[b] <= v < V-1
            = sum_{j=nv-1}^{nv*C-1} g[b, j]   for v = V-1

Sharding: data-parallel over batch across 8 cores, W replicated.

Device strategy (v5): everything is fetched with large packed dma_gather
calls (int16 indices -> vocab split into 4x32768-row buckets; np.unique
gives sorted uniques so buckets are contiguous runs), round-robined over
4 SWDGE queues so descriptor generation runs on all Q7 core pairs in
parallel (~3x faster than one queue). The queue pattern must stay
lane-consistent with Tile's 8-lane DMASW semaphore rotation, so ALL
SWDGE DMAs are dma_gather pieces with queue_num = emission_index % 4.

Bag sums: for slot s,  sum_u CNT[u, s] * rsqrt(max(1, |W_u|^2)) * W_u
over per-core UNIQUE bag codes; one 128x32 @ 128xE fp32 matmul per
128-row chunk accumulates in PSUM. Singles (individual visit rows) are
deduped per core, gathered bucket-sorted, renormalized, and stored as a
contiguous block; the host unpermutes them into (b, v) slots (pad rows
stay zero by construction).
"""

import os

import numpy as np

import concourse.bacc as bacc
import concourse.bass as bass
import concourse.mybir as mybir
import concourse.tile as tile
from concourse.bass_utils import run_bass_kernel_spmd

P = 128          # SBUF partitions
N_CORES = 8
GCH = 24         # chunks per compute block
BUCK = 1 << 15   # vocab rows per dma_gather bucket (int16 index range)
CH_G = 1024      # max idxs per dma_gather piece (64 descs/engine packet limit)
NQ_G = 4         # SWDGE queues, round-robined in emission order

LAST_RESULTS = None   # test harness reads profiling info from here


def _bucketize(uniq, bmax, NBUCK):
    """Split sorted unique ids into per-bucket segments padded to sizes bmax
    (each a multiple of 128). Returns (flat_local_ids, pos_of_uniq)."""
    NB = int(bmax.sum())
    flat = np.zeros(NB, np.int64)
    pos = np.full(len(uniq), -1, np.int64)
    offs = np.concatenate([[0], np.cumsum(bmax)]).astype(np.int64)
    for j in range(NBUCK):
        m = (uniq >= j * BUCK) & (uniq < (j + 1) * BUCK)
        seg = uniq[m]
        o = int(offs[j])
        flat[o : o + len(seg)] = seg - j * BUCK
        pos[np.where(m)[0]] = o + np.arange(len(seg))
    return flat, pos, offs


def _prepare(codes, nv, B, V, C, L, VOCAB, E):
    """Host-side index/count construction. Returns static structure + per-core data."""
    B_LOC = B // N_CORES
    NBUCK = (VOCAB + BUCK - 1) // BUCK
    nbag = nv * (C - 1) + 1                    # bag length per patient

    # balanced LPT assignment: sort desc by bag length, give to least-loaded core
    order = np.argsort(-nbag, kind="stable")
    loads = np.zeros(N_CORES, dtype=np.int64)
    counts = np.zeros(N_CORES, dtype=np.int64)
    assign = np.zeros((N_CORES, B_LOC), dtype=np.int64)
    for b in order:
        k = min((kk for kk in range(N_CORES) if counts[kk] < B_LOC),
                key=lambda kk: loads[kk])
        assign[k, counts[k]] = b
        counts[k] += 1
        loads[k] += nbag[b]

    # ---- per-core unique code sets ----
    bag_u, bag_cnt, bag_bs = [], [], []
    sg_u, sg_entries, sg_bs = [], [], []
    for k in range(N_CORES):
        bvals, bslots = [], []
        svals, s_sv = [], []
        for s in range(B_LOC):
            b = assign[k, s]
            n = int(nv[b])
            bvals.append(codes[b, n - 1 : n * C])
            bslots.append(np.full(n * (C - 1) + 1, s, dtype=np.int64))
            if n > 1:
                svals.append(codes[b, 0 : n - 1])
                vv = np.arange(V - n, V - 1)       # output visit rows
                s_sv.append(np.stack([np.full(n - 1, s), vv], axis=1))
        bv = np.concatenate(bvals)
        bs = np.concatenate(bslots)
        uniq, inv = np.unique(bv, return_inverse=True)   # sorted -> bucket runs
        cnt = np.zeros((len(uniq), B_LOC), np.float32)
        np.add.at(cnt, (inv, bs), 1.0)
        bag_u.append(uniq)
        bag_cnt.append(cnt)
        bag_bs.append(np.bincount(uniq // BUCK, minlength=NBUCK))

        sv = np.concatenate(svals) if svals else np.zeros(0, np.int64)
        se = np.concatenate(s_sv) if s_sv else np.zeros((0, 2), np.int64)
        su, sinv = np.unique(sv, return_inverse=True)
        sg_u.append(su)
        sg_entries.append((se, sinv))              # (s, v) rows + unique idx
        sg_bs.append(np.bincount(su // BUCK, minlength=NBUCK))

    r128 = lambda x: -(-x // P) * P
    SSb = np.array([r128(max(bs[j] for bs in sg_bs)) for j in range(NBUCK)],
                   dtype=np.int64)
    SB = np.array([r128(max(bs[j] for bs in bag_bs)) for j in range(NBUCK)],
                  dtype=np.int64)
    NS, NB = int(SSb.sum()), int(SB.sum())
    T_S, T_BAG = NS // P, NB // P
    T_ALL = T_S + T_BAG

    idx16 = np.zeros((N_CORES, P, (NS + NB) // 16), np.int16)
    CNT = np.zeros((N_CORES, P, T_BAG * B_LOC), np.float32)
    # host-side unpermute info: full[bvec, vvec] = singles_block[pvec, cvec]
    unperm = []

    for k in range(N_CORES):
        fsg, spos, _ = _bucketize(sg_u[k], SSb, NBUCK)
        fbag, _, _ = _bucketize(bag_u[k], SB, NBUCK)
        flat = np.concatenate([fsg, fbag])
        wrap = flat.astype(np.int16).reshape((NS + NB) // 16, 16).T
        idx16[k] = np.tile(wrap, (8, 1))

        cp = np.zeros((NB, B_LOC), np.float32)
        _, bpos, _ = _bucketize(bag_u[k], SB, NBUCK)
        cp[bpos] = bag_cnt[k]
        CNT[k] = cp.reshape(T_BAG, P, B_LOC).transpose(1, 0, 2).reshape(
            P, T_BAG * B_LOC
        )

        se, sinv = sg_entries[k]
        i_flat = spos[sinv]                        # flat gather position
        unperm.append((
            assign[k][se[:, 0]],                   # patient ids
            se[:, 1],                              # visit rows
            i_flat % P,                            # partition
            i_flat // P,                           # chunk
        ))

    # gather pieces: (global chunk offset, num idxs, in-bucket row base)
    pieces = []
    goff = 0
    for (sizes) in (SSb, SB):
        for j in range(NBUCK):
            nj = int(sizes[j])
            for o in range(0, nj, CH_G):
                pc = min(CH_G, nj - o)
                pieces.append((goff + o, pc, j))
            goff += nj

    return dict(
        B_LOC=B_LOC, T_S=T_S, T_BAG=T_BAG, T_ALL=T_ALL, NBUCK=NBUCK,
        pieces=pieces, assign=assign, idx16=idx16, CNT=CNT, unperm=unperm,
    )


def _build(prep, V, C, VOCAB, E):
    """Emit the Bass/Tile program (shared across all 8 cores)."""
    B_LOC, T_S, T_BAG, T_ALL = (
        prep["B_LOC"], prep["T_S"], prep["T_BAG"], prep["T_ALL"]
    )
    pieces = prep["pieces"]
    f32 = mybir.dt.float32
    i16 = mybir.dt.int16
    OUT_ROWS = B_LOC + P * T_S

    nc = bacc.Bacc("TRN2", num_devices=N_CORES, debug=False,
                   num_swdge_queues=NQ_G)
    W_d = nc.dram_tensor("W", [VOCAB, E], f32, kind="ExternalInput")
    idx16_d = nc.dram_tensor("idx16", [P, (T_ALL * P) // 16], i16,
                             kind="ExternalInput")
    cnt_d = nc.dram_tensor("CNT", [P, T_BAG * B_LOC], f32, kind="ExternalInput")
    out_d = nc.dram_tensor("out", [OUT_ROWS, E], f32, kind="ExternalOutput")

    # compute blocks: singles first (their data arrives first), then bag
    blocks = [(0, T_S)]
    c = T_S
    while c < T_ALL:
        blocks.append((c, min(c + GCH, T_ALL)))
        c = min(c + GCH, T_ALL)

    with tile.TileContext(nc) as tc:
        with (
            tc.tile_pool(name="const", bufs=1) as cpool,
            tc.tile_pool(name="g", bufs=1) as gpool,
            tc.tile_pool(name="sq", bufs=2) as sqpool,
            tc.tile_pool(name="sm", bufs=2) as smpool,
            tc.tile_pool(name="ps", bufs=1, space="PSUM") as pspool,
        ):
            idx16_t = cpool.tile_from(idx16_d[:])
            cnt_t = cpool.tile_from(cnt_d[:])

            g = gpool.tile([P, T_ALL * E], f32, tag="g", bufs=1)
            n_t = smpool.tile([P, T_ALL], f32, tag="n", bufs=1)
            rn = smpool.tile([P, T_ALL], f32, tag="rn", bufs=1)
            M = gpool.tile([P, T_BAG * B_LOC], f32, tag="M", bufs=1)
            psum = pspool.tile([B_LOC, E], f32)

            # zero bias tile written by DVE so ACT waits only on DVE
            zbias = smpool.tile([P, 1], f32, tag="zbias", bufs=1)
            nc.vector.memset(zbias[:], 0.0)

            # ---- gathers: packed dma_gather pieces, RR over SWDGE queues.
            # queue_num must equal emission_index % NQ_G so Tile's 8-lane
            # DMASW sem rotation stays queue-consistent per lane.
            for i, (o, pc, j) in enumerate(pieces):
                nrows = min(VOCAB, (j + 1) * BUCK) - j * BUCK
                nc.gpsimd.dma_gather(
                    out_ap=g[:, (o // P) * E : ((o + pc) // P) * E]
                    .rearrange("p (c e) -> p c e", e=E),
                    in_ap=W_d[j * BUCK : j * BUCK + nrows],
                    idxs_ap=idx16_t[:, o // 16 : (o + pc) // 16],
                    num_idxs=pc,
                    num_idxs_reg=pc,
                    elem_size=E,
                    single_packet=True,
                    queue_num=i % NQ_G,
                )

            # ---- per-block compute ----
            for (c0, c1) in blocks:
                w = c1 - c0
                sq = sqpool.tile([P, max(GCH, T_S) * E], f32, tag="sq")
                nc.scalar.activation(
                    sq[:, : w * E], g[:, c0 * E : c1 * E],
                    mybir.ActivationFunctionType.Square, bias=zbias[:],
                )
                nc.vector.tensor_reduce(
                    n_t[:, c0:c1],
                    sq[:, : w * E].rearrange("p (c e) -> p c e", e=E),
                    axis=mybir.AxisListType.X, op=mybir.AluOpType.add,
                )
                nc.vector.tensor_scalar_max(n_t[:, c0:c1], n_t[:, c0:c1], 1.0)
                nc.scalar.activation(
                    rn[:, c0:c1], n_t[:, c0:c1],
                    mybir.ActivationFunctionType.Sqrt, bias=zbias[:],
                )
                nc.vector.reciprocal(rn[:, c0:c1], rn[:, c0:c1])

                if c0 == 0:
                    # singles: renormalize in place, store contiguous block
                    nc.vector.tensor_tensor(
                        out=g[:, : T_S * E].rearrange("p (c e) -> p c e", e=E),
                        in0=g[:, : T_S * E].rearrange("p (c e) -> p c e", e=E),
                        in1=rn[:, 0:T_S].to_broadcast([P, T_S, E]),
                        op=mybir.AluOpType.mult,
                    )
                    nc.sync.dma_start(
                        out=out_d[B_LOC:].rearrange("(p c) e -> p c e", c=T_S),
                        in_=g[:, : T_S * E].rearrange("p (c e) -> p c e", e=E),
                    )
                else:
                    t0, t1 = c0 - T_S, c1 - T_S
                    nc.vector.tensor_tensor(
                        out=M[:, t0 * B_LOC : t1 * B_LOC].rearrange(
                            "p (c s) -> p c s", s=B_LOC
                        ),
                        in0=cnt_t[:, t0 * B_LOC : t1 * B_LOC].rearrange(
                            "p (c s) -> p c s", s=B_LOC
                        ),
                        in1=rn[:, c0:c1].to_broadcast([P, w, B_LOC]),
                        op=mybir.AluOpType.mult,
                    )
                    for t in range(t0, t1):
                        nc.tensor.matmul(
                            out=psum[:, :],
                            lhsT=M[:, t * B_LOC : (t + 1) * B_LOC],
                            rhs=g[:, (T_S + t) * E : (T_S + t + 1) * E],
                            start=(t == 0),
                            stop=(t == T_BAG - 1),
                        )

            outS = smpool.tile([B_LOC, E], f32, tag="outS", bufs=1)
            nc.vector.tensor_copy(outS[:], psum[:])
            nc.sync.dma_start(out=out_d[:B_LOC], in_=outS[:])

    nc.compile()
    return nc


def kernel(**inputs) -> np.ndarray:
    global LAST_RESULTS
    W = np.ascontiguousarray(np.asarray(inputs["W"], dtype=np.float32))
    codes_in = np.asarray(inputs["codes"])
    nv = np.asarray(inputs["n_visits"]).astype(np.int64)

    B, V, C = codes_in.shape
    VOCAB, E = W.shape
    L = V * C
    codes = np.ascontiguousarray(codes_in.reshape(B, L).astype(np.int32))

    prep = _prepare(codes, nv, B, V, C, L, VOCAB, E)
    nc = _build(prep, V, C, VOCAB, E)

    in_maps = [
        {"W": W, "idx16": prep["idx16"][k], "CNT": prep["CNT"][k]}
        for k in range(N_CORES)
    ]
    trace = bool(int(os.environ.get("KERNEL_TRACE", "0")))
    res = run_bass_kernel_spmd(
        nc, in_maps, core_ids=list(range(N_CORES)), trace=trace
    )
    LAST_RESULTS = res

    B_LOC, T_S = prep["B_LOC"], prep["T_S"]
    assign = prep["assign"]
    full = np.zeros((B, V, E), np.float32)
    for k in range(N_CORES):
        o = res.results[k]["out"]
        full[assign[k], V - 1] = o[:B_LOC]
        sing = o[B_LOC:].reshape(P, T_S, E)
        bvec, vvec, pvec, cvec = prep["unperm"][k]
        full[bvec, vvec] = sing[pvec, cvec]
    return full
